# revision 1
# baseline (speedup 1.0000x reference)
"""Multi-head attention (B=4, S=2048, D=768, H=12, d=64) on 8 trn2 NeuronCores.

Sharding: core c handles batch b = c//2 and head-group g = c%2 (6 heads each).
Per core: column-parallel QKV projections (wq/wk/wv column slices), full
attention for its 6 heads, row-parallel output projection (wo row slice).
The two partial outputs per batch are reduced on the host (+ bo and the
bv @ wo correction, exact because softmax rows sum to 1).

Device layout: everything is computed in "feature-on-partition" space.
Inputs are fed pre-transposed (XT = X.T, [768, 2048]) so the contraction
dim of every matmul is on partitions. Matmuls run in float32r (full PE
rate at N>=256, ~7e-4 absmax error vs fp32). Softmax skips the max
subtraction (scores ~ N(0, 0.3), no overflow risk) and the row sums
(over the partition dim) are accumulated on the DVE and reduced with a
ones-vector matmul on the PE.
"""
import sys

for _p in ("/opt/trn_rl_repo", "/root/.axon_site/_ro/trn_rl_repo"):
    if _p not in sys.path:
        sys.path.append(_p)

import numpy as np

import concourse.bass as bass  # noqa: F401  (engine namespaces live on the nc object)
import concourse.bacc as bacc
import concourse.mybir as mybir
import concourse.tile as tile
from concourse.bass_utils import run_bass_kernel_spmd

B, S, D = 4, 2048, 768
NUM_HEADS, HEAD = 12, 64
NCORES = 8
HPC = NUM_HEADS // 2          # 6 heads per core
MC = HPC * HEAD               # 384 per-core projection cols
KT = D // 128                 # 6 contraction k-tiles
MT = MC // 128                # 3 head-pair tiles
ST = S // 128                 # 16 sequence tiles
SQW = 512                     # sq chunk width (one PSUM bank)
SQC = S // SQW                # 4 sq chunks

F32 = mybir.dt.float32
F32R = mybir.dt.float32r
EXP = mybir.ActivationFunctionType.Exp
ADD = mybir.AluOpType.add
MULT = mybir.AluOpType.mult

_NC = None
LAST_RESULTS = None
_LAST_IN_MAPS = None  # for test harnesses: BassKernelResults of the last run


def _build(loop=None):
    nc = bacc.Bacc("TRN2", target_bir_lowering=False, debug=False,
                   num_devices=NCORES)
    xqt = nc.declare_dram_parameter("xqt", [D, S], F32R, isOutput=False)
    xkt = nc.declare_dram_parameter("xkt", [D, S], F32R, isOutput=False)
    xvt = nc.declare_dram_parameter("xvt", [D, S], F32R, isOutput=False)
    wq = nc.declare_dram_parameter("wq", [D, MC], F32R, isOutput=False)
    wk = nc.declare_dram_parameter("wk", [D, MC], F32R, isOutput=False)
    wv = nc.declare_dram_parameter("wv", [D, MC], F32R, isOutput=False)
    wo = nc.declare_dram_parameter("wo", [MC, D], F32R, isOutput=False)
    bq = nc.declare_dram_parameter("bq", [MC], F32, isOutput=False)
    bk = nc.declare_dram_parameter("bk", [MC], F32, isOutput=False)
    cst_d = nc.declare_dram_parameter("cst", [128, 226], F32R, isOutput=False)
    out = nc.declare_dram_parameter("out", [S, D], F32, isOutput=True)

    with tile.TileContext(nc) as tc:
        if loop:
            with tc.For_i(0, loop, 1):
                _emit(nc, tc, xqt, xkt, xvt, wq, wk, wv, wo, bq, bk, cst_d, out)
        else:
            _emit(nc, tc, xqt, xkt, xvt, wq, wk, wv, wo, bq, bk, cst_d, out)
    nc.compile()
    return nc


def _emit(nc, tc, xqt, xkt, xvt, wq, wk, wv, wo, bq, bk, cst_d, out):
    ctx_lp = nc.allow_low_precision(reason="float32r tiles feed the PE; accumulation stays fp32 in PSUM")
    ctx_lp.__enter__()
    with (
        tc.tile_pool(name="qtp", bufs=MT) as qt_pool,
        tc.tile_pool(name="ktp", bufs=MT) as kt_pool,
        tc.tile_pool(name="vp", bufs=ST) as v_pool,
        tc.tile_pool(name="cst", bufs=1) as cst_pool,
    ):
        # constant lhsT patterns (see kernel() for the host-side layout):
        # [:,0:1]=ones  [:,1:34]=[32 zero cols|ones]  [:,34:98]=ones
        # [:,98:226]=[64 zero cols|64 one cols]
        cst = cst_pool.tile([128, 226], F32R, tag="cst")
        bq_sb = cst_pool.tile([128, MT], F32, tag="bq")
        bk_sb = cst_pool.tile([128, MT], F32, tag="bk")

        qt = [qt_pool.tile([128, S], F32R, tag="qt", name=f"qt{m}") for m in range(MT)]
        kt = [kt_pool.tile([128, S], F32R, tag="kt", name=f"kt{m}") for m in range(MT)]
        # per pair hp: cols [0:64]=V_even, [64:128]=zeros, [128:192]=V_odd
        vt = [v_pool.tile([128, MT, 3 * HEAD], F32R, tag="v", name=f"vt{st}") for st in range(ST)]

        # ---------------- Phase A: projections ----------------
        # x^T is loaded in column chunks (double-buffered) so the next
        # input's DMA overlaps this input's matmuls.
        ACW = 1024
        NAC = S // ACW             # 4 column chunks per input
        with (
            tc.tile_pool(name="xtp", bufs=3) as xt_pool,
            tc.tile_pool(name="wp", bufs=1) as w_pool,
            tc.tile_pool(name="psA", bufs=4, space="PSUM") as psA,
            tc.tile_pool(name="psV", bufs=3, space="PSUM") as psV,
        ):
            w_sb = {}
            for name, w in (("wv", wv), ("wq", wq), ("wk", wk)):
                w_sb[name] = w_pool.tile([128, KT, MC], F32R, tag=name, name=f"w_{name}")
                nc.sync.dma_start(
                    out=w_sb[name], in_=w[:].rearrange("(n k) m -> k n m", k=128))
            # constants/biases are not needed until mid-attention; keep them
            # off the critical first-matmul DMA path
            nc.sync.dma_start(out=cst, in_=cst_d[:])
            nc.sync.dma_start(out=bq_sb, in_=bq[:].rearrange("(t p) -> p t", p=128))
            nc.sync.dma_start(out=bk_sb, in_=bk[:].rearrange("(t p) -> p t", p=128))

            for x_dram, wname, dst, bias_sb in ((xvt, "wv", None, None),
                                                (xqt, "wq", qt, bq_sb),
                                                (xkt, "wk", kt, bk_sb)):
                for c in range(NAC):
                    cols = slice(c * ACW, (c + 1) * ACW)
                    x_sb = xt_pool.tile([128, KT, ACW], F32R, tag="xt")
                    for t in range(KT):
                        nc.sync.dma_start(out=x_sb[:, t],
                                          in_=x_dram[t * 128:(t + 1) * 128, cols])
                    if dst is not None:
                        # QT/KT[m*128+p, s] = sum_k W[k, m*128+p] * XT[k, s] + b
                        for m in range(MT):
                            for h in range(ACW // SQW):
                                ps = psA.tile([128, SQW], F32, tag="psA")
                                for k in range(KT):
                                    nc.tensor.matmul(
                                        ps,
                                        w_sb[wname][:, k, m * 128:(m + 1) * 128],
                                        x_sb[:, k, h * SQW:(h + 1) * SQW],
                                        start=(k == 0), stop=(k == KT - 1))
                                s0 = c * ACW + h * SQW
                                nc.vector.tensor_scalar_add(
                                    dst[m][:, s0:s0 + SQW], ps,
                                    bias_sb[:, m:m + 1])
                    else:
                        # V[st*128+p, m] = sum_k XvT[k, st*128+p] * Wv[k, m]
                        for st8 in range(ACW // 128):
                            st = c * (ACW // 128) + st8
                            ps = psV.tile([128, MC], F32, tag="psV")
                            for k in range(KT):
                                nc.tensor.matmul(
                                    ps,
                                    x_sb[:, k, st8 * 128:(st8 + 1) * 128],
                                    w_sb["wv"][:, k, :],
                                    start=(k == 0), stop=(k == KT - 1))
                            psv = ps.rearrange("p (t two d) -> p t two d", two=2, d=HEAD)
                            nc.vector.tensor_copy(vt[st][:, :, 0:HEAD], psv[:, :, 0])
                            nc.vector.tensor_copy(vt[st][:, :, 2 * HEAD:], psv[:, :, 1])
                            nc.vector.tensor_scalar_mul(
                                vt[st][:, :, HEAD:2 * HEAD], psv[:, :, 0], 0.0)

        # ---------------- Phase B+C: attention + output projection ----------
        # CW=1024 sq chunks: scores/ctx matmuls in 512 halves (PSUM bank
        # limit) but exp and the denominator adds run 1024 wide. ctx
        # accumulators are [128,512] half tiles with bufs=3 so the next
        # chunk starts while the previous one normalizes. One shared
        # [128,512] PSUM slot serves rowsum/broadcast/out-projection.
        CW = 1024
        NCH = S // CW              # 2 chunks
        GP_OPS = 15                # e1-adds handled by GPSIMD per chunk
        with (
            tc.tile_pool(name="ctxp", bufs=MT) as ctx_pool,
            tc.tile_pool(name="ep", bufs=2) as e_pool,
            tc.tile_pool(name="accp", bufs=2) as acc_pool,
            tc.tile_pool(name="rp", bufs=2) as r_pool,
            tc.tile_pool(name="wop", bufs=1) as wo_pool,
            tc.tile_pool(name="outp", bufs=4) as out_pool,
            tc.tile_pool(name="psS", bufs=2, space="PSUM") as psS,
            tc.tile_pool(name="psC", bufs=3, space="PSUM") as psC,
            tc.tile_pool(name="psM", bufs=1, space="PSUM") as psM,
        ):
            ctx = [ctx_pool.tile([128, S], F32R, tag="ctx", name=f"ctx{m}") for m in range(MT)]
            wo_sb = wo_pool.tile([128, MT, D], F32R, tag="wo")
            nc.sync.dma_start(out=wo_sb,
                              in_=wo[:].rearrange("(t p) o -> p t o", p=128))

            def emit_norm_reduce(state):
                # stage 1: partition-reduce matmuls + reciprocals
                sc, hp, ps_ch, acc0, acc1a = state
                rr = []
                for h4 in range(CW // SQW):
                    qs = slice(h4 * SQW, (h4 + 1) * SQW)
                    ps_r = psM.tile([33, SQW], F32, tag="psM", name=f"psr{sc}{hp}{h4}")
                    nc.tensor.matmul(ps_r, cst[:, 1:34], acc1a[:, qs],
                                     start=True, stop=False,
                                     skip_group_check=True)
                    nc.tensor.matmul(ps_r[0:1, :], cst[:, 0:1], acc0[:, qs],
                                     start=False, stop=True,
                                     skip_group_check=True)
                    r0 = r_pool.tile([1, SQW], F32R, tag="r0")
                    r1 = r_pool.tile([1, SQW], F32R, tag="r1")
                    nc.vector.reciprocal(r0, ps_r[0:1, :])
                    nc.vector.reciprocal(r1, ps_r[32:33, :])
                    rr.append((r0, r1))
                return rr

            def emit_norm_bcast(state, rr, h4):
                # stage 2: broadcast matmuls + normalize into ctx (one half)
                sc, hp, ps_ch, acc0, acc1a = state
                if True:
                    r0, r1 = rr[h4]
                    ps_b = psM.tile([128, SQW], F32, tag="psM", name=f"psb{sc}{hp}{h4}")
                    nc.tensor.matmul(ps_b, cst[0:1, 98:226], r1,
                                     start=True, stop=False,
                                     skip_group_check=True)
                    nc.tensor.matmul(ps_b[0:64, :], cst[0:1, 34:98], r0,
                                     start=False, stop=True,
                                     skip_group_check=True)
                    b_sb = r_pool.tile([128, SQW], F32, tag="bsb", bufs=2)
                    nc.vector.tensor_copy(b_sb, ps_b)
                    s0 = sc * CW + h4 * SQW
                    nc.vector.tensor_tensor(ctx[hp][:, s0:s0 + SQW],
                                            ps_ch[h4], b_sb, op=MULT)

            def outproj_rounds(sc, at_tail):
                # one round = one [128,512 or 256] PSUM tile of out rows;
                # yielded so the caller can spread rounds across sk slots.
                # At the tail, alternate psM/psC slots so rounds pipeline.
                for st4 in range(CW // 128):
                    s0 = sc * CW + st4 * 128
                    o_sb = out_pool.tile([128, D], F32, tag="osb")
                    for i, (n0, nw) in enumerate(((0, 512), (512, 256))):
                        pool = psM if (not at_tail) or (st4 * 2 + i) % 4 == 0 else psC
                        ps_o = pool.tile([128, 512], F32,
                                         tag="psC" if pool is psC else "psM",
                                         name=f"pso{sc}{st4}{n0}")
                        for m in range(MT):
                            nc.tensor.matmul(
                                ps_o[:, 0:nw],
                                ctx[m][:, s0:s0 + 128],
                                wo_sb[:, m, n0:n0 + nw],
                                start=(m == 0), stop=(m == MT - 1))
                        if at_tail:
                            nc.scalar.copy(o_sb[:, n0:n0 + nw], ps_o[:, 0:nw])
                        else:
                            nc.vector.tensor_copy(o_sb[:, n0:n0 + nw], ps_o[:, 0:nw])
                        if i == 1:
                            nc.sync.dma_start(out=out[s0:s0 + 128, :], in_=o_sb)
                        yield

            pending = None          # finished chunk awaiting normalize
            pending_out = None      # sc whose out-proj is due
            for sc in range(NCH):
                for hp in range(MT):
                    ps_ch = []
                    acc0 = acc_pool.tile([128, CW], F32R, tag="acc0")
                    acc1a = acc_pool.tile([128, CW], F32R, tag="acc1a")

                    def emit_ctx(sk, e0, e1, sc=sc, hp=hp):
                        # odd head: zero-padded [128,128] lhsT (fp32r has no
                        # col tiling); goes first with start=True at sk==0
                        if not ps_ch:
                            for h in range(CW // SQW):
                                ps_ch.append(psC.tile([128, SQW], F32, tag="psC",
                                                      name=f"psc{sc}_{hp}_{h}"))
                        for h4 in range(CW // SQW):
                            qs = slice(h4 * SQW, (h4 + 1) * SQW)
                            nc.tensor.matmul(ps_ch[h4], vt[sk][:, hp, HEAD:],
                                             e1[:, qs], start=(sk == 0),
                                             stop=False, skip_group_check=True)
                            nc.tensor.matmul(ps_ch[h4][0:64, :], vt[sk][:, hp, 0:HEAD],
                                             e0[:, qs], start=False,
                                             stop=(sk == ST - 1),
                                             skip_group_check=True)

                    # software pipeline: ctx matmuls trail scores/exp by one
                    # sk; the previous chunk's normalize + out-proj are
                    # emitted two sk-iterations in so the PE queue never
                    # heads with work that waits on ACT/DVE/GPSIMD tails.
                    prev = None
                    for sk in range(ST):
                        sks = slice(sk * 128, (sk + 1) * 128)
                        ps_s0 = psS.tile([128, CW], F32, tag="psS")
                        ps_s1 = psS.tile([128, CW], F32, tag="psS")
                        e0 = e_pool.tile([128, CW], F32R, tag="e0", bufs=5)
                        e1 = e_pool.tile([128, CW], F32R, tag="e1", bufs=6)
                        for h4 in range(CW // SQW):
                            sq = slice(sc * CW + h4 * SQW, sc * CW + (h4 + 1) * SQW)
                            qs = slice(h4 * SQW, (h4 + 1) * SQW)
                            nc.tensor.matmul(ps_s0[:, qs], kt[hp][0:64, sks],
                                             qt[hp][0:64, sq])
                            nc.tensor.matmul(ps_s1[:, qs], kt[hp][64:128, sks],
                                             qt[hp][64:128, sq])
                        nc.scalar.activation(e0, ps_s0, EXP, scale=0.125)
                        nc.scalar.activation(e1, ps_s1, EXP, scale=0.125)
                        if prev is not None:
                            emit_ctx(*prev)
                        # denominator partials: acc0 on DVE; acc1 mostly on
                        # GPSIMD (~2x slower per op) with the final add done
                        # on DVE so the slower engine never gates the chunk.
                        # Chains start with a 2-input add of the first two e
                        # tiles (no init copy) and the last e1 folds straight
                        # into acc1a (no separate merge).
                        if sk == 1:
                            nc.vector.tensor_tensor(acc0, prev[1], e0, op=ADD)
                            nc.gpsimd.tensor_tensor(acc1a, prev[2], e1, op=ADD)
                        elif sk >= 2 and sk < GP_OPS:
                            nc.vector.tensor_tensor(acc0, acc0, e0, op=ADD)
                            nc.gpsimd.tensor_tensor(acc1a, acc1a, e1, op=ADD)
                        elif sk >= GP_OPS:
                            nc.vector.tensor_tensor(acc0, acc0, e0, op=ADD)
                            nc.vector.tensor_tensor(acc1a, acc1a, e1, op=ADD)
                        prev = (sk, e0, e1)
                        if sk == 3 and pending is not None:
                            pending_rr = emit_norm_reduce(pending)
                        if sk == 5 and pending is not None:
                            emit_norm_bcast(pending, pending_rr, 0)
                        if sk == 7 and pending is not None:
                            emit_norm_bcast(pending, pending_rr, 1)
                            pending = None
                        if sk >= 10 and pending_out is not None:
                            if next(pending_out, StopIteration) is StopIteration:
                                pending_out = None
                    emit_ctx(*prev)
                    pending = (sc, hp, ps_ch, acc0, acc1a)
                if sc < NCH - 1:
                    pending_out = outproj_rounds(sc, at_tail=False)
            # tail: interleave the final normalize halves with the
            # out-proj rounds that only depend on the already-done half
            rr_last = emit_norm_reduce(pending)
            emit_norm_bcast(pending, rr_last, 0)
            emit_norm_bcast(pending, rr_last, 1)
            for _ in outproj_rounds(NCH - 1, at_tail=True):
                pass


def _cst_host():
    c = np.zeros((128, 226), np.float32)
    c[:, 0] = 1.0      # M=1 ones reduce column
    c[:, 33] = 1.0     # row 32 of the zero-padded M=33 reduce
    c[:, 34:98] = 1.0  # [1,64] broadcast ones
    c[:, 162:226] = 1.0  # [1,128] zero-padded broadcast (rows 64:128)
    return c


def kernel(query, key, value, wq, bq, wk, bk, wv, bv, wo, bo):
    global _NC, LAST_RESULTS, _LAST_IN_MAPS
    if _NC is None:
        _NC = _build()

    def f32c(a):
        return np.ascontiguousarray(np.asarray(a, dtype=np.float32))

    query, key, value = map(np.asarray, (query, key, value))
    xt = [{"xqt": f32c(query[b].T), "xkt": f32c(key[b].T),
           "xvt": f32c(value[b].T)} for b in range(B)]
    wslices = []
    for g in range(2):
        cols = slice(g * MC, (g + 1) * MC)
        wslices.append({
            "wq": f32c(np.asarray(wq)[:, cols]),
            "wk": f32c(np.asarray(wk)[:, cols]),
            "wv": f32c(np.asarray(wv)[:, cols]),
            "wo": f32c(np.asarray(wo)[cols, :]),
            "bq": f32c(np.asarray(bq)[cols]),
            "bk": f32c(np.asarray(bk)[cols]),
            "cst": _cst_host(),
        })
    in_maps = [dict(xt[c // 2], **wslices[c % 2]) for c in range(NCORES)]

    global _LAST_IN_MAPS
    _LAST_IN_MAPS = in_maps
    res = run_bass_kernel_spmd(_NC, in_maps, core_ids=list(range(NCORES)))
    LAST_RESULTS = res

    # host epilogue: pairwise partial-sum reduce + biases (bv@wo is exact
    # because softmax rows sum to 1, so ctx absorbs bv additively)
    corr = (np.asarray(bv, np.float64) @ np.asarray(wo, np.float64)
            + np.asarray(bo, np.float64)).astype(np.float32)
    y = np.empty((B, S, D), np.float32)
    for b in range(B):
        y[b] = res.results[2 * b]["out"] + res.results[2 * b + 1]["out"] + corr
    return y

